# revision 90
# baseline (speedup 1.0000x reference)
"""Trainium2 Bass kernel for nn_CCM_77043123355820 (complex conv mask, single frame).

Math (reference): m (1,27,1,257) -> complex 3x3 mask H via basis V;
xc = concat(cache, x) along time; out[f] = sum_{kt,kf} M[kt,kf,f] * X[kt, f+kf-1]
(complex); new_cache = xc[:, :, 1:].

Raw-Bass implementation (the walrus build in this container accepts only ONE
embedded semaphore wait per instruction, so sync is explicit standalone
wait_ge instructions; the Tile layer's attached multi-waits don't compile).

Mapping (F=257 on the free axis, 9 taps on partitions 0-8; every SBUF operand
is base-partition 0 -- compute ops require equal 32-aligned base partitions):
  mx input [9,1285]: host-marshalled layout prep: cols 0-770 = the three m
  coefficient groups c=0,1,2 (k-major), cols 771-1284 = zero-padded tap
  windows xw[kt*3+kf, f(+257)] = X[kt, f+kf-1] (re|im). One load DMA --
  each DMA costs ~2.3us latency (625ns serialized HWDGE slot + 650ns DGE
  delay + 900ns sem propagation), so input marshalling is folded on host.
  Device compute:
    hr = m0 - 0.5*(m9+m18), hd = m9-m18 (sqrt(3)/2 folds into the final
    scalar_tensor_tensor ops);
    gA = hr (broadcast x2) * xw on DVE; gB = hd * xw halves on Pool (the
    independent imag branch runs on the gpsimd engine in parallel);
    s[:,0:257]   = gA_re - 0.866*gB_im   (x_enh real, per tap)
    s[:,257:514] = gA_im + 0.866*gB_re   (x_enh imag, per tap);
    s is written as bf16 (single rounding of the final per-tap terms,
    ~2e-3 scale-relative error) so the two 9-partition reduction matmuls
    run at 1 cycle/row (this walrus build rejects fp32r matmuls; fp32
    would be 4 cycles/row) -> pq [1,1024] PSUM fp32 (bank 0 = real,
    bank 1 = imag). The imag half is produced in two column chunks so its
    matmul and interleave-copy pipeline behind the s_im ops; DVE copies
    the imag chunks and ACT copies the real half into osb (f, ri)
    interleaved, so the y store is one contiguous DMA.
  new_cache is written by two DRAM->DRAM DMAs straight from the inputs.
  Output DMAs are fenced by an SP DGE drain (the stock BassBlock teardown
  fence) instead of a semaphore wait, and the stock exit barrier is
  replaced by branch bookkeeping only (_LeanBlock).

Sharding: B=1 single stream -> no useful intra-op sharding (per spec hint);
the identical tiny program runs SPMD on all 8 cores, core 0's output returned.
"""

import numpy as np

import concourse.bass as bass
import concourse.mybir as mybir
from concourse.bass_utils import run_bass_kernel_spmd

F = 257
N_CORES = 8
SQ32 = float(np.sqrt(3.0) / 2.0)
MXW = 3 * F + 2 * F  # 1285

_NC = None


class _LeanBlock(bass.BassBlock):
    """BassBlock whose exit only does the branch bookkeeping: the kernel's
    output DMAs are already fenced by the explicit SP DGE drain, so the
    stock per-engine drains + all-engine barrier are pure tail latency."""

    def __enter__(self):
        assert self.bass.cur_block is None
        self.bass.cur_block = self
        return super().__enter__()

    def __exit__(self, exc_type, exc_val, exc_tb):
        self.bass.cur_block = None
        if exc_type is not None:
            return
        for engine, last_body in self.last_body.items():
            with self.bass.body(
                last_body, parent=self.bass.cur_bb, allow_existing_parent=True
            ):
                engine.br(self.end_bb)
        self.bass.switch_bb(self.end_bb)


def _build_nc():
    f32 = mybir.dt.float32
    bf16 = mybir.dt.bfloat16
    op = mybir.AluOpType
    nc = bass.Bass(enable_partition_id=False, monotonic_sem_count=0)

    mx_d = nc.dram_tensor("mx", [9, MXW], f32, kind="ExternalInput")
    x_d = nc.dram_tensor("x", [F, 2], f32, kind="ExternalInput")
    c_d = nc.dram_tensor("cache", [F, 2, 2], f32, kind="ExternalInput")
    y_d = nc.dram_tensor("y", [F, 2], f32, kind="ExternalOutput")
    o_d = nc.dram_tensor("ncache", [F, 2, 2], f32, kind="ExternalOutput")

    with (
        nc.sbuf_tensor("mx_t", [9, MXW], f32) as mx,
        nc.sbuf_tensor("u", [9, F], f32) as u,
        nc.sbuf_tensor("hr", [9, F], f32) as hr,
        nc.sbuf_tensor("hd", [9, F], f32) as hd,
        nc.sbuf_tensor("ga", [9, 2 * F], f32) as ga,
        nc.sbuf_tensor("gb", [9, 2 * F], f32) as gb,
        nc.sbuf_tensor("s", [9, 2 * F], bf16) as s,
        nc.sbuf_tensor("ones", [9, 1], bf16) as ones,
        nc.sbuf_tensor("dmy", [9, 2], bf16) as dmy,
        nc.sbuf_tensor("warm", [1, 1], bf16) as warm,
        nc.sbuf_tensor("osb", [1, 2 * F], f32) as osb,
        nc.psum_tensor("pq", [1, 1024], f32) as pq,
        nc.psum_tensor("pscr", [1, 2], f32) as pscr,
        nc.semaphore("in_sem") as in_sem,
        nc.semaphore("out_sem") as out_sem,
        nc.semaphore("dve_sem") as dve_sem,
        nc.semaphore("pe_sem") as pe_sem,
        nc.semaphore("pool_sem") as pool_sem,
        nc.semaphore("act_sem") as act_sem,
        _LeanBlock(nc, f"b{nc.next_id()}") as block,
    ):
        ma = mx[:, 0:F]
        mb = mx[:, F : 2 * F]
        mc = mx[:, 2 * F : 3 * F]
        xw = mx[:, 3 * F : MXW]
        xw_re = mx[:, 3 * F : 4 * F]
        xw_im = mx[:, 4 * F : 5 * F]

        @block.sync
        def _(sync):
            sync.dma_start(mx[:], mx_d[:]).then_inc(in_sem, 16)
            # new_cache[f,0,:] = cache[f,1,:]; new_cache[f,1,:] = x[f,:]
            sync.dma_start(o_d[:, 0, :], c_d[:, 1, :]).then_inc(out_sem, 16)
            sync.dma_start(o_d[:, 1, :], x_d[:, :]).then_inc(out_sem, 16)
            # y store once DVE (imag chunks) + ACT (real) filled osb
            sync.wait_ge(act_sem, 1)
            sync.dma_start(y_d[:, :], osb[:])._wait_ge(dve_sem, 6).then_inc(
                out_sem, 16
            )
            # DGE drain fences all SP-issued output DMAs before teardown
            # (the same teardown fence the stock BassBlock tail emits)
            sync.drain()

        @block.vector
        def _(vector):
            vector.memset(ones[:], 1.0)
            vector.memset(dmy[:], 1.0).then_inc(dve_sem, 1)
            # ---- hr = m0 - 0.5*(m9+m18) ----
            vector.wait_ge(in_sem, 16)
            vector.scalar_tensor_tensor(u[:], mb, -0.5, ma, op.mult, op.add)
            vector.scalar_tensor_tensor(hr[:], mc, -0.5, u[:], op.mult, op.add)
            # ---- gA = hr*(xw re|im); Pool computes gB halves ----
            hr_b2 = bass.AP(hr, 0, [[F, 9], [0, 2], [1, F]])
            vector.tensor_mul(ga[:], hr_b2, xw)
            # ---- signed sums (0.866 folded in) ----
            vector.wait_ge(pool_sem, 2)
            vector.scalar_tensor_tensor(
                s[:, 0:F], gb[:, F : 2 * F], -SQ32, ga[:, 0:F], op.mult, op.add
            ).then_inc(dve_sem, 1)
            # s_im in two column chunks so mm_im/copy_im pipeline behind it
            vector.wait_ge(pool_sem, 3)
            vector.scalar_tensor_tensor(
                s[:, F : F + 128],
                gb[:, 0:128],
                SQ32,
                ga[:, F : F + 128],
                op.mult,
                op.add,
            ).then_inc(dve_sem, 1)
            vector.scalar_tensor_tensor(
                s[:, F + 128 : 2 * F],
                gb[:, 128:F],
                SQ32,
                ga[:, F + 128 : 2 * F],
                op.mult,
                op.add,
            ).then_inc(dve_sem, 1)
            # ---- interleave imag chunks (f, ri=1) into osb; ACT does real
            vector.wait_ge(pe_sem, 2)
            vector.tensor_copy(
                bass.AP(osb, 1, [[2 * F, 1], [2, 128]]),
                pq[:, 512:640],
            ).then_inc(dve_sem, 1)
            vector.wait_ge(pe_sem, 3)
            vector.tensor_copy(
                bass.AP(osb, 257, [[2 * F, 1], [2, 129]]),
                pq[:, 640 : 640 + 129],
            ).then_inc(dve_sem, 1)

        @block.scalar
        def _(scalar):
            # warm copy preloads the ACT Copy path during the DMA window
            scalar.wait_ge(dve_sem, 1)
            scalar.copy(warm[:], ones[0:1, 0:1])
            # interleave PE real half (f, ri=0) into osb
            scalar.wait_ge(pe_sem, 1)
            scalar.copy(
                bass.AP(osb, 0, [[2 * F, 1], [2, F]]),
                pq[:, 0:F],
            ).then_inc(act_sem, 1)

        @block.gpsimd
        def _(gpsimd):
            # ---- hd = m9 - m18; gB = hd*(xw im|re), imag half first ----
            gpsimd.wait_ge(in_sem, 16)
            gpsimd.tensor_sub(hd[:], mb, mc).then_inc(pool_sem, 1)
            gpsimd.tensor_mul(gb[:, F : 2 * F], hd[:], xw_im).then_inc(pool_sem, 1)
            gpsimd.tensor_mul(gb[:, 0:F], hd[:], xw_re).then_inc(pool_sem, 1)

        @block.tensor
        def _(tensor):
            # early dummy matmul: starts the PE p-state ramp during the
            # input-DMA window so the real reductions run at full clock
            tensor.wait_ge(dve_sem, 1)
            tensor.matmul(pscr[:], ones[:], dmy[:])
            # partition reduction: out[1,257] = ones[9,1].T @ s-half; the imag
            # half runs as two column chunks pipelined behind the s_im ops
            tensor.wait_ge(dve_sem, 2)
            tensor.matmul(pq[:, 0:F], ones[:], s[:, 0:F]).then_inc(pe_sem, 1)
            tensor.wait_ge(dve_sem, 3)
            tensor.matmul(
                pq[:, 512:640], ones[:], s[:, F : F + 128]
            ).then_inc(pe_sem, 1)
            tensor.wait_ge(dve_sem, 4)
            tensor.matmul(
                pq[:, 640 : 640 + 129], ones[:], s[:, F + 128 : 2 * F]
            ).then_inc(pe_sem, 1)

    return nc


def _in_map(m: np.ndarray, x: np.ndarray, cache: np.ndarray) -> dict:
    m27 = np.ascontiguousarray(np.asarray(m).reshape(27, F), np.float32)
    x2 = np.ascontiguousarray(np.asarray(x).reshape(F, 2), np.float32)
    c3 = np.ascontiguousarray(np.asarray(cache).reshape(F, 2, 2), np.float32)

    mx = np.zeros((9, MXW), np.float32)
    # m coefficient groups, k-major: mx[k, c*257+f] = m27[c*9+k, f]
    mx[:, 0 : 3 * F] = (
        m27.reshape(3, 9, F).transpose(1, 0, 2).reshape(9, 3 * F)
    )
    # zero-padded frames: xpad[t(re)/3+t(im), 1+f]
    xpad = np.zeros((6, F + 2), np.float32)
    xpad[0:2, 1 : 1 + F] = c3[:, :, 0].T
    xpad[3:5, 1 : 1 + F] = c3[:, :, 1].T
    xpad[2, 1 : 1 + F] = x2[:, 0]
    xpad[5, 1 : 1 + F] = x2[:, 1]
    # tap windows: xw[kt*3+kf, f(+257)] = xpad[kt(+3), kf+f]
    for kt in range(3):
        for kf in range(3):
            k = kt * 3 + kf
            mx[k, 3 * F : 4 * F] = xpad[kt, kf : kf + F]
            mx[k, 4 * F : 5 * F] = xpad[3 + kt, kf : kf + F]
    return {"mx": mx, "x": x2, "cache": c3}


def kernel(m: np.ndarray, x: np.ndarray, cache: np.ndarray):
    global _NC
    if _NC is None:
        _NC = _build_nc()

    res = run_bass_kernel_spmd(
        _NC, [_in_map(m, x, cache)] * N_CORES, list(range(N_CORES))
    ).results[0]
    y = np.asarray(res["y"], np.float32).reshape(1, F, 1, 2)
    ncache = np.asarray(res["ncache"], np.float32).reshape(1, F, 2, 2)
    return y, ncache


# revision 95
# speedup vs baseline: 1.0038x; 1.0038x over previous
"""Trainium2 Bass kernel for nn_CCM_77043123355820 (complex conv mask, single frame).

Math (reference): m (1,27,1,257) -> complex 3x3 mask H via basis V;
xc = concat(cache, x) along time; out[f] = sum_{kt,kf} M[kt,kf,f] * X[kt, f+kf-1]
(complex); new_cache = xc[:, :, 1:].

Raw-Bass implementation (the walrus build in this container accepts only ONE
embedded semaphore wait per instruction, so sync is explicit standalone
wait_ge instructions; the Tile layer's attached multi-waits don't compile).

Mapping (F=257 on the free axis, 9 taps on partitions 0-8; every SBUF operand
is base-partition 0 -- compute ops require equal 32-aligned base partitions):
  mx input [9,1285]: host-marshalled layout prep: cols 0-770 = the three m
  coefficient groups c=0,1,2 (k-major), cols 771-1284 = zero-padded tap
  windows xw[kt*3+kf, f(+257)] = X[kt, f+kf-1] (re|im). Loaded as two DMAs
  (m part first: its smaller transfer starts the serial H-prep sooner; the
  tap windows land just before the product ops need them). Each DMA costs
  ~2.3us latency (625ns serialized HWDGE slot + 650ns DGE delay + 900ns
  sem propagation), so input marshalling is folded on host.
  Device compute:
    hr = m0 - 0.5*(m9+m18), hd = m9-m18 (sqrt(3)/2 folds into the final
    scalar_tensor_tensor ops);
    gA = hr (broadcast x2) * xw on DVE; gB = hd * xw halves on Pool (the
    independent imag branch runs on the gpsimd engine in parallel);
    s[:,0:257]   = gA_re - 0.866*gB_im   (x_enh real, per tap)
    s[:,257:514] = gA_im + 0.866*gB_re   (x_enh imag, per tap);
    s is written as bf16 (single rounding of the final per-tap terms,
    ~2e-3 scale-relative error) so the two 9-partition reduction matmuls
    run at 1 cycle/row (this walrus build rejects fp32r matmuls; fp32
    would be 4 cycles/row) -> pq [1,1024] PSUM fp32 (bank 0 = real,
    bank 1 = imag). The imag half is produced in two column chunks so its
    matmul and interleave-copy pipeline behind the s_im ops; DVE copies
    the imag chunks and ACT copies the real half into osb (f, ri)
    interleaved, so the y store is one contiguous DMA.
  new_cache is written by two DRAM->DRAM DMAs straight from the inputs.
  Output DMAs are fenced by an SP DGE drain (the stock BassBlock teardown
  fence) instead of a semaphore wait, and the stock exit barrier is
  replaced by branch bookkeeping only (_LeanBlock).

Sharding: B=1 single stream -> no useful intra-op sharding (per spec hint);
the identical tiny program runs SPMD on all 8 cores, core 0's output returned.
"""

import numpy as np

import concourse.bass as bass
import concourse.mybir as mybir
from concourse.bass_utils import run_bass_kernel_spmd

F = 257
N_CORES = 8
SQ32 = float(np.sqrt(3.0) / 2.0)
MXW = 3 * F + 2 * F  # 1285

_NC = None


class _LeanBlock(bass.BassBlock):
    """BassBlock whose exit only does the branch bookkeeping: the kernel's
    output DMAs are already fenced by the explicit SP DGE drain, so the
    stock per-engine drains + all-engine barrier are pure tail latency."""

    def __enter__(self):
        assert self.bass.cur_block is None
        self.bass.cur_block = self
        return super().__enter__()

    def __exit__(self, exc_type, exc_val, exc_tb):
        self.bass.cur_block = None
        if exc_type is not None:
            return
        for engine, last_body in self.last_body.items():
            with self.bass.body(
                last_body, parent=self.bass.cur_bb, allow_existing_parent=True
            ):
                engine.br(self.end_bb)
        self.bass.switch_bb(self.end_bb)


def _build_nc():
    f32 = mybir.dt.float32
    bf16 = mybir.dt.bfloat16
    op = mybir.AluOpType
    nc = bass.Bass(enable_partition_id=False, monotonic_sem_count=0)

    mx_d = nc.dram_tensor("mx", [9, MXW], f32, kind="ExternalInput")
    x_d = nc.dram_tensor("x", [F, 2], f32, kind="ExternalInput")
    c_d = nc.dram_tensor("cache", [F, 2, 2], f32, kind="ExternalInput")
    y_d = nc.dram_tensor("y", [F, 2], f32, kind="ExternalOutput")
    o_d = nc.dram_tensor("ncache", [F, 2, 2], f32, kind="ExternalOutput")

    with (
        nc.sbuf_tensor("mx_t", [9, MXW], f32) as mx,
        nc.sbuf_tensor("u", [9, F], f32) as u,
        nc.sbuf_tensor("hr", [9, F], f32) as hr,
        nc.sbuf_tensor("hd", [9, F], f32) as hd,
        nc.sbuf_tensor("ga", [9, 2 * F], f32) as ga,
        nc.sbuf_tensor("gb", [9, 2 * F], f32) as gb,
        nc.sbuf_tensor("s", [9, 2 * F], bf16) as s,
        nc.sbuf_tensor("ones", [9, 1], bf16) as ones,
        nc.sbuf_tensor("dmy", [9, 2], bf16) as dmy,
        nc.sbuf_tensor("warm", [1, 1], bf16) as warm,
        nc.sbuf_tensor("osb", [1, 2 * F], f32) as osb,
        nc.psum_tensor("pq", [1, 1024], f32) as pq,
        nc.psum_tensor("pscr", [1, 2], f32) as pscr,
        nc.semaphore("in_sem") as in_sem,
        nc.semaphore("xw_sem") as xw_sem,
        nc.semaphore("out_sem") as out_sem,
        nc.semaphore("dve_sem") as dve_sem,
        nc.semaphore("pe_sem") as pe_sem,
        nc.semaphore("pool_sem") as pool_sem,
        nc.semaphore("act_sem") as act_sem,
        _LeanBlock(nc, f"b{nc.next_id()}") as block,
    ):
        ma = mx[:, 0:F]
        mb = mx[:, F : 2 * F]
        mc = mx[:, 2 * F : 3 * F]
        xw = mx[:, 3 * F : MXW]
        xw_re = mx[:, 3 * F : 4 * F]
        xw_im = mx[:, 4 * F : 5 * F]

        @block.sync
        def _(sync):
            # m coefficients first (smaller transfer, feeds the serial
            # H-prep); tap windows second
            sync.dma_start(mx[:, 0 : 3 * F], mx_d[:, 0 : 3 * F]).then_inc(
                in_sem, 16
            )
            sync.dma_start(
                mx[:, 3 * F : MXW], mx_d[:, 3 * F : MXW]
            ).then_inc(xw_sem, 16)
            # new_cache[f,0,:] = cache[f,1,:]; new_cache[f,1,:] = x[f,:]
            sync.dma_start(o_d[:, 0, :], c_d[:, 1, :]).then_inc(out_sem, 16)
            sync.dma_start(o_d[:, 1, :], x_d[:, :]).then_inc(out_sem, 16)
            # y store once DVE (imag chunks) + ACT (real) filled osb
            sync.wait_ge(act_sem, 1)
            sync.dma_start(y_d[:, :], osb[:])._wait_ge(dve_sem, 6).then_inc(
                out_sem, 16
            )
            # DGE drain fences all SP-issued output DMAs before teardown
            # (the same teardown fence the stock BassBlock tail emits)
            sync.drain()

        @block.vector
        def _(vector):
            vector.memset(ones[:], 1.0)
            vector.memset(dmy[:], 1.0).then_inc(dve_sem, 1)
            # ---- hr = m0 - 0.5*(m9+m18) ----
            vector.wait_ge(in_sem, 16)
            vector.scalar_tensor_tensor(u[:], mb, -0.5, ma, op.mult, op.add)
            vector.scalar_tensor_tensor(hr[:], mc, -0.5, u[:], op.mult, op.add)
            # ---- gA = hr*(xw re|im); Pool computes gB halves ----
            hr_b2 = bass.AP(hr, 0, [[F, 9], [0, 2], [1, F]])
            vector.wait_ge(xw_sem, 16)
            vector.tensor_mul(ga[:], hr_b2, xw)
            # ---- signed sums (0.866 folded in) ----
            vector.wait_ge(pool_sem, 2)
            vector.scalar_tensor_tensor(
                s[:, 0:F], gb[:, F : 2 * F], -SQ32, ga[:, 0:F], op.mult, op.add
            ).then_inc(dve_sem, 1)
            # s_im in two column chunks so mm_im/copy_im pipeline behind it
            vector.wait_ge(pool_sem, 3)
            vector.scalar_tensor_tensor(
                s[:, F : F + 128],
                gb[:, 0:128],
                SQ32,
                ga[:, F : F + 128],
                op.mult,
                op.add,
            ).then_inc(dve_sem, 1)
            vector.scalar_tensor_tensor(
                s[:, F + 128 : 2 * F],
                gb[:, 128:F],
                SQ32,
                ga[:, F + 128 : 2 * F],
                op.mult,
                op.add,
            ).then_inc(dve_sem, 1)
            # ---- interleave imag chunks (f, ri=1) into osb; ACT does real
            vector.wait_ge(pe_sem, 2)
            vector.tensor_copy(
                bass.AP(osb, 1, [[2 * F, 1], [2, 128]]),
                pq[:, 512:640],
            ).then_inc(dve_sem, 1)
            vector.wait_ge(pe_sem, 3)
            vector.tensor_copy(
                bass.AP(osb, 257, [[2 * F, 1], [2, 129]]),
                pq[:, 640 : 640 + 129],
            ).then_inc(dve_sem, 1)

        @block.scalar
        def _(scalar):
            # warm copy preloads the ACT Copy path during the DMA window
            scalar.wait_ge(dve_sem, 1)
            scalar.copy(warm[:], ones[0:1, 0:1])
            # interleave PE real half (f, ri=0) into osb
            scalar.wait_ge(pe_sem, 1)
            scalar.copy(
                bass.AP(osb, 0, [[2 * F, 1], [2, F]]),
                pq[:, 0:F],
            ).then_inc(act_sem, 1)

        @block.gpsimd
        def _(gpsimd):
            # ---- hd = m9 - m18; gB = hd*(xw im|re), imag half first ----
            gpsimd.wait_ge(in_sem, 16)
            gpsimd.tensor_sub(hd[:], mb, mc).then_inc(pool_sem, 1)
            gpsimd.wait_ge(xw_sem, 16)
            gpsimd.tensor_mul(gb[:, F : 2 * F], hd[:], xw_im).then_inc(pool_sem, 1)
            gpsimd.tensor_mul(gb[:, 0:F], hd[:], xw_re).then_inc(pool_sem, 1)

        @block.tensor
        def _(tensor):
            # early dummy matmul: starts the PE p-state ramp during the
            # input-DMA window so the real reductions run at full clock
            tensor.wait_ge(dve_sem, 1)
            tensor.matmul(pscr[:], ones[:], dmy[:])
            # partition reduction: out[1,257] = ones[9,1].T @ s-half; the imag
            # half runs as two column chunks pipelined behind the s_im ops
            tensor.wait_ge(dve_sem, 2)
            tensor.matmul(pq[:, 0:F], ones[:], s[:, 0:F]).then_inc(pe_sem, 1)
            tensor.wait_ge(dve_sem, 3)
            tensor.matmul(
                pq[:, 512:640], ones[:], s[:, F : F + 128]
            ).then_inc(pe_sem, 1)
            tensor.wait_ge(dve_sem, 4)
            tensor.matmul(
                pq[:, 640 : 640 + 129], ones[:], s[:, F + 128 : 2 * F]
            ).then_inc(pe_sem, 1)

    return nc


def _in_map(m: np.ndarray, x: np.ndarray, cache: np.ndarray) -> dict:
    m27 = np.ascontiguousarray(np.asarray(m).reshape(27, F), np.float32)
    x2 = np.ascontiguousarray(np.asarray(x).reshape(F, 2), np.float32)
    c3 = np.ascontiguousarray(np.asarray(cache).reshape(F, 2, 2), np.float32)

    mx = np.zeros((9, MXW), np.float32)
    # m coefficient groups, k-major: mx[k, c*257+f] = m27[c*9+k, f]
    mx[:, 0 : 3 * F] = (
        m27.reshape(3, 9, F).transpose(1, 0, 2).reshape(9, 3 * F)
    )
    # zero-padded frames: xpad[t(re)/3+t(im), 1+f]
    xpad = np.zeros((6, F + 2), np.float32)
    xpad[0:2, 1 : 1 + F] = c3[:, :, 0].T
    xpad[3:5, 1 : 1 + F] = c3[:, :, 1].T
    xpad[2, 1 : 1 + F] = x2[:, 0]
    xpad[5, 1 : 1 + F] = x2[:, 1]
    # tap windows: xw[kt*3+kf, f(+257)] = xpad[kt(+3), kf+f]
    for kt in range(3):
        for kf in range(3):
            k = kt * 3 + kf
            mx[k, 3 * F : 4 * F] = xpad[kt, kf : kf + F]
            mx[k, 4 * F : 5 * F] = xpad[3 + kt, kf : kf + F]
    return {"mx": mx, "x": x2, "cache": c3}


def kernel(m: np.ndarray, x: np.ndarray, cache: np.ndarray):
    global _NC
    if _NC is None:
        _NC = _build_nc()

    res = run_bass_kernel_spmd(
        _NC, [_in_map(m, x, cache)] * N_CORES, list(range(N_CORES))
    ).results[0]
    y = np.asarray(res["y"], np.float32).reshape(1, F, 1, 2)
    ncache = np.asarray(res["ncache"], np.float32).reshape(1, F, 2, 2)
    return y, ncache
